# revision 5
# baseline (speedup 1.0000x reference)
"""Trainium2 Bass kernel for nn_AtomicLinear: out = x @ W.T + bias.

Shapes (hardcoded): x (4096, 2048) f32, weight (2048, 2048) f32 [out, in],
bias (2048,) f32 -> out (4096, 2048) f32.

Sharding across 8 NeuronCores: 2D grid of 4 batch-groups x 2 out-feature
groups. Core c handles batch rows [bg*1024, (bg+1)*1024) and out features
[og*1024, (og+1)*1024) with bg = c // 2, og = c % 2. Per-core HBM traffic is
8 MB (x^T shard) + 8 MB (W^T shard) + 0.5 MB (bias bcast) + 4 MB (out) ~=
20.5 MB -- less than pure data-parallel (24 MB), balanced against the
~58 us fp32r TensorE time (true ridge).

The TensorE contracts over the partition dim, so both operands need
in_features on partitions; fp32 has no DMA-transpose path, so the wrapper
marshals x^T / W^T (host-side layout choice during sharding) and the device
kernel is a pure fp32r matmul + bias add:

  psum[b(128), o(512)] += sum_k xT[k, b-slice] (lhsT) @ wT[k, o-slice] (rhs)

fp32r streams 1 row/cycle at free-dim >= 256 (vs 4 cycles/row for exact
fp32), accumulating in fp32 PSUM. Measured steady-state: ~227 ns per
[128x128]x[128x512] matmul.

Schedule notes (from NTFF profiles):
- The contraction is split into two k-halves per output tile. With a
  single 16-chunk accumulation, the 8 PSUM banks cap concurrency at 8 of
  the 16 output tiles and the PE starves while input chunks stream in
  (~2.56 us/chunk vs 1.8 us of bank-eligible work). With k-halves, the
  bank is recycled after 8 chunks and every chunk has 16 eligible
  matmuls (~3.6 us) -- the PE never starves.
- A short warmup of junk matmuls runs during the DMA prologue so the
  PE_HAM clock gate reaches 8/8 before the first real matmul (cold
  matmuls run at half rate for the first ~3.4 us of activity).
- bias rides the ACT HWDGE ring so the SP ring delivers chunk 0 sooner.
"""

import numpy as np

BATCH = 4096
IN_F = 2048
OUT_F = 2048
N_CORES = 8
BG = 4  # batch groups
OG = 2  # out-feature groups
B_SH = BATCH // BG  # 1024 batch rows per core
O_SH = OUT_F // OG  # 1024 out features per core
P = 128
N_TILE = 512
K_TILES = IN_F // P  # 16
K_HALF = K_TILES // 2  # 8
M_TILES = B_SH // P  # 8
N_TILES = O_SH // N_TILE  # 2
WARMUP_MMS = 12

_BUILT = None  # cached compiled program -- neuronx compile is expensive


def _build():
    import concourse.mybir as mybir
    import concourse.tile as tile
    from concourse import bacc

    nc = bacc.Bacc(None, target_bir_lowering=False, debug=False)

    xT = nc.declare_dram_parameter("xT", [IN_F, B_SH], mybir.dt.float32r, isOutput=False)
    wT = nc.declare_dram_parameter("wT", [IN_F, O_SH], mybir.dt.float32r, isOutput=False)
    bias_b = nc.declare_dram_parameter("bias_b", [P, O_SH], mybir.dt.float32, isOutput=False)
    out = nc.declare_dram_parameter("out", [B_SH, O_SH], mybir.dt.float32, isOutput=True)

    with tile.TileContext(nc) as tc:
        with (
            tc.tile_pool(name="persist", bufs=1) as persist,
            tc.tile_pool(name="part_pool", bufs=16) as part_pool,
            tc.tile_pool(name="ot_pool", bufs=4) as ot_pool,
            tc.tile_pool(name="ps_pool", bufs=8, space="PSUM") as ps_pool,
        ):
            # bias on the ACT HWDGE ring (input chunks own the SP ring).
            bias_sb = persist.tile([P, O_SH], mybir.dt.float32, name="bias_sb", tag="bias_sb")
            nc.scalar.dma_start(out=bias_sb[:], in_=bias_b[:])

            # PE warmup: junk matmuls with no DMA dependency keep the PE
            # busy through the HAM activity window during the DMA prologue.
            # (memset is not a legal walrus instruction on float32r, so the
            # tile is float32 and the matmul operands are bitcast views.)
            junk = persist.tile([P, P + N_TILE], mybir.dt.float32, name="junk", tag="junk")
            nc.vector.memset(junk[:], 0.0)
            junk_r = junk[:].bitcast(mybir.dt.float32r)
            warm_ps = ps_pool.tile([P, N_TILE], mybir.dt.float32, name="warm_ps", tag="ps")
            for i in range(WARMUP_MMS):
                nc.tensor.matmul(
                    warm_ps[:],
                    junk_r[:, :P],
                    junk_r[:, P:],
                    start=(i == 0),
                    stop=(i == WARMUP_MMS - 1),
                )

            # One SBUF tile per 128-row k-chunk of W; x k-chunks are split
            # into m-halves (lo = m0..3, hi = m4..7). Emission order per
            # k-half-phase: [w_k + x_k_lo] pairs first (768 KB -> ~1.9 us
            # cadence, matching the 8-bank-limited ~1.8 us of eligible PE
            # work per chunk), then the x_hi halves. Everything stays
            # resident (16 MB < 24 MB SBUF).
            wk = [
                persist.tile([P, O_SH], mybir.dt.float32r, name=f"wk{k}", tag=f"wk{k}")
                for k in range(K_TILES)
            ]
            xk_lo = [
                persist.tile([P, B_SH // 2], mybir.dt.float32r, name=f"xkl{k}", tag=f"xkl{k}")
                for k in range(K_TILES)
            ]
            xk_hi = [
                persist.tile([P, B_SH // 2], mybir.dt.float32r, name=f"xkh{k}", tag=f"xkh{k}")
                for k in range(K_TILES)
            ]

            def xslice(k, m):
                half, off = (xk_lo, 0) if m < M_TILES // 2 else (xk_hi, B_SH // 2)
                return half[k][:, m * P - off : (m + 1) * P - off]

            for kh in range(2):
                ks = range(kh * K_HALF, (kh + 1) * K_HALF)
                for k in ks:
                    nc.sync.dma_start(out=wk[k][:], in_=wT[k * P : (k + 1) * P, :])
                    nc.sync.dma_start(out=xk_lo[k][:], in_=xT[k * P : (k + 1) * P, : B_SH // 2])
                for k in ks:
                    nc.sync.dma_start(out=xk_hi[k][:], in_=xT[k * P : (k + 1) * P, B_SH // 2 :])

            # Phase 0: accumulate k-chunks 0..7 for every output tile,
            # evict partial+bias to SBUF (frees the PSUM bank after 8 chunks).
            parts = {}
            for m in range(M_TILES):
                for n in range(N_TILES):
                    pt = ps_pool.tile([P, N_TILE], mybir.dt.float32, name=f"psA_{m}_{n}", tag="ps")
                    for k in range(K_HALF):
                        nc.tensor.matmul(
                            pt[:],
                            xslice(k, m),
                            wk[k][:, n * N_TILE : (n + 1) * N_TILE],
                            start=(k == 0),
                            stop=(k == K_HALF - 1),
                        )
                    part = part_pool.tile(
                        [P, N_TILE], mybir.dt.float32, name=f"part_{m}_{n}", tag="part"
                    )
                    nc.vector.tensor_add(
                        out=part[:],
                        in0=pt[:],
                        in1=bias_sb[:, n * N_TILE : (n + 1) * N_TILE],
                    )
                    parts[(m, n)] = part

            # Phase 1: accumulate k-chunks 8..15, add the stored partial,
            # store to DRAM via the ACT HWDGE ring.
            for m in range(M_TILES):
                for n in range(N_TILES):
                    pt = ps_pool.tile([P, N_TILE], mybir.dt.float32, name=f"psB_{m}_{n}", tag="ps")
                    for k in range(K_HALF, K_TILES):
                        nc.tensor.matmul(
                            pt[:],
                            xslice(k, m),
                            wk[k][:, n * N_TILE : (n + 1) * N_TILE],
                            start=(k == K_HALF),
                            stop=(k == K_TILES - 1),
                        )
                    ot = ot_pool.tile([P, N_TILE], mybir.dt.float32, name=f"ot_{m}_{n}", tag="ot")
                    nc.vector.tensor_add(out=ot[:], in0=pt[:], in1=parts[(m, n)][:])
                    nc.scalar.dma_start(
                        out=out[m * P : (m + 1) * P, n * N_TILE : (n + 1) * N_TILE],
                        in_=ot[:],
                    )

    nc.compile()
    return nc


def _get_built():
    global _BUILT
    if _BUILT is None:
        _BUILT = _build()
    return _BUILT


def _make_in_maps(x, weight, bias):
    x = np.ascontiguousarray(x, dtype=np.float32)
    weight = np.ascontiguousarray(weight, dtype=np.float32)
    bias = np.ascontiguousarray(bias, dtype=np.float32)

    xT_q = [np.ascontiguousarray(x[bg * B_SH : (bg + 1) * B_SH, :].T) for bg in range(BG)]
    wT_h = [np.ascontiguousarray(weight[og * O_SH : (og + 1) * O_SH, :].T) for og in range(OG)]
    bias_bc = [
        np.ascontiguousarray(np.broadcast_to(bias[og * O_SH : (og + 1) * O_SH], (P, O_SH)))
        for og in range(OG)
    ]

    in_maps = []
    for c in range(N_CORES):
        bg, og = c // OG, c % OG
        in_maps.append({"xT": xT_q[bg], "wT": wT_h[og], "bias_b": bias_bc[og]})
    return in_maps


def _assemble(results):
    full = np.empty((BATCH, OUT_F), dtype=np.float32)
    for c in range(N_CORES):
        bg, og = c // OG, c % OG
        full[bg * B_SH : (bg + 1) * B_SH, og * O_SH : (og + 1) * O_SH] = results[c]["out"]
    return full


def _run(inputs, trace=False, **spmd_kwargs):
    """Run the SPMD kernel; returns (full_output, BassKernelResults)."""
    from concourse.bass_utils import run_bass_kernel_spmd

    nc = _get_built()
    in_maps = _make_in_maps(inputs["x"], inputs["weight"], inputs["bias"])
    res = run_bass_kernel_spmd(nc, in_maps, list(range(N_CORES)), trace=trace, **spmd_kwargs)
    return _assemble(res.results), res


def kernel(x, weight, bias):
    out, _ = _run({"x": x, "weight": weight, "bias": bias})
    return out


# revision 6
# speedup vs baseline: 1.1735x; 1.1735x over previous
"""Trainium2 Bass kernel for nn_AtomicLinear: out = x @ W.T + bias.

Shapes (hardcoded): x (4096, 2048) f32, weight (2048, 2048) f32 [out, in],
bias (2048,) f32 -> out (4096, 2048) f32.

Sharding across 8 NeuronCores: 2D grid of 4 batch-groups x 2 out-feature
groups. Core c handles batch rows [bg*1024, (bg+1)*1024) and out features
[og*1024, (og+1)*1024) with bg = c // 2, og = c % 2. Per-core HBM traffic is
8 MB (x^T shard) + 8 MB (W^T shard) + 0.5 MB (bias bcast) + 4 MB (out) ~=
20.5 MB -- less than pure data-parallel (24 MB), balanced against the
~58 us fp32r TensorE time (true ridge).

The TensorE contracts over the partition dim, so both operands need
in_features on partitions; fp32 has no DMA-transpose path, so the wrapper
marshals x^T / W^T (host-side layout choice during sharding) and the device
kernel is a pure fp32r matmul + bias add:

  psum[b(128), o(512)] += sum_k xT[k, b-slice] (lhsT) @ wT[k, o-slice] (rhs)

fp32r streams 1 row/cycle at free-dim >= 256 (vs 4 cycles/row for exact
fp32), accumulating in fp32 PSUM. Measured steady-state: ~227 ns per
[128x128]x[128x512] matmul.

Schedule notes (from NTFF profiles):
- The contraction is split into two k-halves per output tile. With a
  single 16-chunk accumulation, the 8 PSUM banks cap concurrency at 8 of
  the 16 output tiles and the PE starves while input chunks stream in
  (~2.56 us/chunk vs 1.8 us of bank-eligible work). With k-halves, the
  bank is recycled after 8 chunks and every chunk has 16 eligible
  matmuls (~3.6 us) -- the PE never starves.
- A short warmup of junk matmuls runs during the DMA prologue so the
  PE_HAM clock gate reaches 8/8 before the first real matmul (cold
  matmuls run at half rate for the first ~3.4 us of activity).
- bias rides the ACT HWDGE ring so the SP ring delivers chunk 0 sooner.
"""

import numpy as np

BATCH = 4096
IN_F = 2048
OUT_F = 2048
N_CORES = 8
BG = 4  # batch groups
OG = 2  # out-feature groups
B_SH = BATCH // BG  # 1024 batch rows per core
O_SH = OUT_F // OG  # 1024 out features per core
P = 128
N_TILE = 512
K_TILES = IN_F // P  # 16
K_HALF = K_TILES // 2  # 8
M_TILES = B_SH // P  # 8
N_TILES = O_SH // N_TILE  # 2
WARMUP_MMS = 10

_BUILT = None  # cached compiled program -- neuronx compile is expensive


def _build():
    import concourse.mybir as mybir
    import concourse.tile as tile
    from concourse import bacc

    nc = bacc.Bacc(None, target_bir_lowering=False, debug=False)

    xT = nc.declare_dram_parameter("xT", [IN_F, B_SH], mybir.dt.float32r, isOutput=False)
    wT = nc.declare_dram_parameter("wT", [IN_F, O_SH], mybir.dt.float32r, isOutput=False)
    bias_b = nc.declare_dram_parameter("bias_b", [P, O_SH], mybir.dt.float32, isOutput=False)
    out = nc.declare_dram_parameter("out", [B_SH, O_SH], mybir.dt.float32, isOutput=True)

    with tile.TileContext(nc) as tc:
        with (
            tc.tile_pool(name="persist", bufs=1) as persist,
            tc.tile_pool(name="part_pool", bufs=16) as part_pool,
            tc.tile_pool(name="ot_pool", bufs=4) as ot_pool,
            tc.tile_pool(name="ps_pool", bufs=8, space="PSUM") as ps_pool,
        ):
            # bias on the ACT HWDGE ring (input chunks own the SP ring).
            bias_sb = persist.tile([P, O_SH], mybir.dt.float32, name="bias_sb", tag="bias_sb")
            nc.scalar.dma_start(out=bias_sb[:], in_=bias_b[:])

            # PE warmup: junk matmuls with no DMA dependency keep the PE
            # busy through the HAM activity window during the DMA prologue.
            # (memset is not a legal walrus instruction on float32r, so the
            # tile is float32 and the matmul operands are bitcast views.)
            junk = persist.tile([P, P + N_TILE], mybir.dt.float32, name="junk", tag="junk")
            nc.vector.memset(junk[:], 0.0)
            junk_r = junk[:].bitcast(mybir.dt.float32r)
            warm_ps = ps_pool.tile([P, N_TILE], mybir.dt.float32, name="warm_ps", tag="ps")
            for i in range(WARMUP_MMS):
                nc.tensor.matmul(
                    warm_ps[:],
                    junk_r[:, :P],
                    junk_r[:, P:],
                    start=(i == 0),
                    stop=(i == WARMUP_MMS - 1),
                )

            # One SBUF tile per 128-row k-chunk of W; x k-chunks are split
            # into m-halves (lo = m0..3, hi = m4..7). Emission order per
            # k-half-phase: [w_k + x_k_lo] pairs first (768 KB -> ~1.9 us
            # cadence, matching the 8-bank-limited ~1.8 us of eligible PE
            # work per chunk), then the x_hi halves. Everything stays
            # resident (16 MB < 24 MB SBUF).
            wk = [
                persist.tile([P, O_SH], mybir.dt.float32r, name=f"wk{k}", tag=f"wk{k}")
                for k in range(K_TILES)
            ]
            xk_lo = [
                persist.tile([P, B_SH // 2], mybir.dt.float32r, name=f"xkl{k}", tag=f"xkl{k}")
                for k in range(K_TILES)
            ]
            xk_hi = [
                persist.tile([P, B_SH // 2], mybir.dt.float32r, name=f"xkh{k}", tag=f"xkh{k}")
                for k in range(K_TILES)
            ]

            def xslice(k, m):
                half, off = (xk_lo, 0) if m < M_TILES // 2 else (xk_hi, B_SH // 2)
                return half[k][:, m * P - off : (m + 1) * P - off]

            for kh in range(2):
                ks = range(kh * K_HALF, (kh + 1) * K_HALF)
                for k in ks:
                    nc.sync.dma_start(out=wk[k][:], in_=wT[k * P : (k + 1) * P, :])
                    nc.sync.dma_start(out=xk_lo[k][:], in_=xT[k * P : (k + 1) * P, : B_SH // 2])
                for k in ks:
                    nc.sync.dma_start(out=xk_hi[k][:], in_=xT[k * P : (k + 1) * P, B_SH // 2 :])

            # Phase 0: accumulate k-chunks 0..7 for every output tile,
            # evict partial+bias to SBUF (frees the PSUM bank after 8 chunks).
            parts = {}
            for m in range(M_TILES):
                for n in range(N_TILES):
                    pt = ps_pool.tile([P, N_TILE], mybir.dt.float32, name=f"psA_{m}_{n}", tag="ps")
                    for k in range(K_HALF):
                        nc.tensor.matmul(
                            pt[:],
                            xslice(k, m),
                            wk[k][:, n * N_TILE : (n + 1) * N_TILE],
                            start=(k == 0),
                            stop=(k == K_HALF - 1),
                        )
                    part = part_pool.tile(
                        [P, N_TILE], mybir.dt.float32, name=f"part_{m}_{n}", tag="part"
                    )
                    nc.vector.tensor_add(
                        out=part[:],
                        in0=pt[:],
                        in1=bias_sb[:, n * N_TILE : (n + 1) * N_TILE],
                    )
                    parts[(m, n)] = part

            # Phase 1: accumulate k-chunks 8..15, add the stored partial,
            # store to DRAM via the ACT HWDGE ring.
            for m in range(M_TILES):
                for n in range(N_TILES):
                    pt = ps_pool.tile([P, N_TILE], mybir.dt.float32, name=f"psB_{m}_{n}", tag="ps")
                    for k in range(K_HALF, K_TILES):
                        nc.tensor.matmul(
                            pt[:],
                            xslice(k, m),
                            wk[k][:, n * N_TILE : (n + 1) * N_TILE],
                            start=(k == K_HALF),
                            stop=(k == K_TILES - 1),
                        )
                    ot = ot_pool.tile([P, N_TILE], mybir.dt.float32, name=f"ot_{m}_{n}", tag="ot")
                    nc.vector.tensor_add(out=ot[:], in0=pt[:], in1=parts[(m, n)][:])
                    nc.scalar.dma_start(
                        out=out[m * P : (m + 1) * P, n * N_TILE : (n + 1) * N_TILE],
                        in_=ot[:],
                    )

    nc.compile()
    return nc


def _get_built():
    global _BUILT
    if _BUILT is None:
        _BUILT = _build()
    return _BUILT


def _make_in_maps(x, weight, bias):
    x = np.ascontiguousarray(x, dtype=np.float32)
    weight = np.ascontiguousarray(weight, dtype=np.float32)
    bias = np.ascontiguousarray(bias, dtype=np.float32)

    xT_q = [np.ascontiguousarray(x[bg * B_SH : (bg + 1) * B_SH, :].T) for bg in range(BG)]
    wT_h = [np.ascontiguousarray(weight[og * O_SH : (og + 1) * O_SH, :].T) for og in range(OG)]
    bias_bc = [
        np.ascontiguousarray(np.broadcast_to(bias[og * O_SH : (og + 1) * O_SH], (P, O_SH)))
        for og in range(OG)
    ]

    in_maps = []
    for c in range(N_CORES):
        bg, og = c // OG, c % OG
        in_maps.append({"xT": xT_q[bg], "wT": wT_h[og], "bias_b": bias_bc[og]})
    return in_maps


def _assemble(results):
    full = np.empty((BATCH, OUT_F), dtype=np.float32)
    for c in range(N_CORES):
        bg, og = c // OG, c % OG
        full[bg * B_SH : (bg + 1) * B_SH, og * O_SH : (og + 1) * O_SH] = results[c]["out"]
    return full


def _run(inputs, trace=False, **spmd_kwargs):
    """Run the SPMD kernel; returns (full_output, BassKernelResults)."""
    from concourse.bass_utils import run_bass_kernel_spmd

    nc = _get_built()
    in_maps = _make_in_maps(inputs["x"], inputs["weight"], inputs["bias"])
    res = run_bass_kernel_spmd(nc, in_maps, list(range(N_CORES)), trace=trace, **spmd_kwargs)
    return _assemble(res.results), res


def kernel(x, weight, bias):
    out, _ = _run({"x": x, "weight": weight, "bias": bias})
    return out
